# Initial kernel scaffold
#
"""Trainium2 Bass kernel for nn_ByteEmbedding (segment_reduce).

Computation (per batch row, one row per NeuronCore, 8 cores):
  byte_emb = emb_weight[x] * sqrt(128)            # gather  [8192, 128]
  grouped  = segment_mean(byte_emb, byte_groups)  # ragged  [2048, 128]
  out      = grouped @ out_proj_w.T               # proj    [2048, 1024]

Device pipeline (scan formulation, transposed layout [dim, pos]):
  1. One-hot vocab rows XohT[v, i] = (x[i] == v) built on DVE/ACT from a
     replicated copy of x (3 chunks of 128 vocab rows).
  2. byte_emb^T = (E*sqrt(128))^T @ XohT on the PE -- the embedding
     "gather" is 3 accumulating matmuls per 512-position slice with the
     128x128 embedding chunks stationary.  No DMA gather at all.
  3. Exclusive prefix sums along positions (tensor_tensor_scan) per
     512-token group window; segment sums are differences of the scan at
     host-precomputed segment boundary positions (ap_gather along the
     free dim).  Both boundaries of every token lie in its group window,
     so the scan carry cancels in the difference and the four group
     scans are fully independent (run on DVE and GpSimd in parallel).
  4. Mean: counts = diff of boundary positions (computed on device from
     the uploaded integer starts vector), reciprocal on the Scalar
     engine (activation table), multiplied into the grouped values.
  5. out = grouped^T.T @ W^T per 128-token tile (W^T built once on the
     PE via transposes).

The group windows are the union over the 8 rows, so one SPMD program
serves all cores; per-core behavior enters only through uploaded integer
index tensors (replicated x, boundary positions).
"""

import os
import sys

import numpy as np

for _p in ("/opt/trn_rl_repo",):
    if _p not in sys.path and os.path.isdir(_p):
        sys.path.append(_p)

import concourse.bacc as bacc
import concourse.bass as bass
import concourse.mybir as mybir
import concourse.tile as tile
from concourse.bass_utils import run_bass_kernel_spmd

B = 8
S = 8192          # bytes per row
V = 384           # vocab (= 3 * 128)
D = 128           # byte dim
E = 1024          # out dim
T = 2048          # tokens
P = 128
NGRP = 4          # token groups of 512
TG = T // NGRP    # 512 tokens per group
NB = TG + 1       # boundaries per group (inclusive)
NBPAD = 528       # padded boundary count (% 16 == 0, % 4 == 0)
SCALE = float(D) ** 0.5
dt = mybir.dt
F32 = dt.float32


def _windows(starts):
    """Union [lo, hi) position window per 512-token group over all rows."""
    w = []
    for g in range(NGRP):
        lo = int(starts[:, TG * g].min())
        hi = int(starts[:, TG * (g + 1)].max())
        w.append((lo, hi))
    return w


def _build(windows) -> bacc.Bacc:
    nc = bacc.Bacc(
        "TRN2",
        target_bir_lowering=False,
        debug=False,
        enable_asserts=True,
        num_devices=B,
    )

    x_rep = nc.dram_tensor("x_rep", [P, S], dt.int16, kind="ExternalInput")
    st_a = nc.dram_tensor("st_a", [P, T // P], dt.int32, kind="ExternalInput")
    st_b = nc.dram_tensor("st_b", [P, T // P], dt.int32, kind="ExternalInput")
    bidxs = [nc.dram_tensor(f"bidx{g}", [P, NBPAD // 16], dt.uint16, kind="ExternalInput")
             for g in range(NGRP)]
    consts = nc.dram_tensor("consts", [P, P + V // P], F32, kind="ExternalInput")
    emb_weight = nc.dram_tensor("emb_weight", [V, D], F32, kind="ExternalInput")
    out_proj_w = nc.dram_tensor("out_proj_w", [E, D], F32, kind="ExternalInput")
    out = nc.dram_tensor("out", [T, E], F32, kind="ExternalOutput")

    with tile.TileContext(nc) as tc:
        with (
            tc.tile_pool(name="sb", bufs=1) as sb,
            tc.tile_pool(name="oh", bufs=6) as oh_pool,
            tc.tile_pool(name="bem", bufs=2) as bem_pool,
            tc.tile_pool(name="sc", bufs=2) as sc_pool,
            tc.tile_pool(name="outsb", bufs=4) as outsb_pool,
            tc.tile_pool(name="ps_g", bufs=1, space="PSUM") as ps_g,
            tc.tile_pool(name="ps_o", bufs=3, space="PSUM") as ps_o,
        ):
            # ---- inputs: per-window slices of the replicated x row ----
            x16_g = []
            for g in range(NGRP):
                lo, hi = windows[g]
                xg = sb.tile([P, hi - lo], dt.int16, name=f"x16_{g}")
                nc.sync.dma_start(out=xg[:], in_=x_rep.ap()[:, lo:hi])
                x16_g.append(xg)

            bidx_ts = []
            for g in range(NGRP):
                bt = sb.tile([P, NBPAD // 16], dt.uint16, name=f"bidx_t{g}")
                nc.sync.dma_start(out=bt[:], in_=bidxs[g].ap())
                bidx_ts.append(bt)

            # ---- reciprocal counts in token-partition layout ----
            # recip_pt[p, c] = 1 / max(starts[128c+p+1] - starts[128c+p], 1);
            # applied after the projection (out rows are tokens there).
            sta_i = sb.tile([P, T // P], dt.int32, name="sta_i")
            nc.sync.dma_start(out=sta_i[:], in_=st_a.ap())
            stb_i = sb.tile([P, T // P], dt.int32, name="stb_i")
            nc.sync.dma_start(out=stb_i[:], in_=st_b.ap())
            sta_f = sb.tile([P, T // P], F32, name="sta_f")
            nc.vector.tensor_copy(out=sta_f[:], in_=sta_i[:])
            stb_f = sb.tile([P, T // P], F32, name="stb_f")
            nc.vector.tensor_copy(out=stb_f[:], in_=stb_i[:])
            cnt_pt = sb.tile([P, T // P], F32, name="cnt_pt")
            nc.vector.tensor_tensor(out=cnt_pt[:], in0=stb_f[:], in1=sta_f[:],
                                    op=mybir.AluOpType.subtract)
            nc.vector.tensor_scalar(out=cnt_pt[:], in0=cnt_pt[:], scalar1=1.0,
                                    scalar2=None, op0=mybir.AluOpType.max)
            recip_pt = sb.tile([P, T // P], F32, name="recip_pt")
            nc.vector.reciprocal(out=recip_pt[:], in_=cnt_pt[:])

            # ---- constants (uploaded: keeps gpsimd gather-only => one
            #      library reload, no per-group reload drains) ----
            consts_t = sb.tile([P, P + V // P], F32, name="consts_t")
            nc.sync.dma_start(out=consts_t[:], in_=consts.ap())
            identity = consts_t[:, 0:P]
            vcols = [consts_t[:, P + v:P + v + 1] for v in range(V // P)]
            zcol = sb.tile([P, 1], F32, name="zcol")
            nc.vector.memset(zcol[:], 0.0)

            # embedding chunks, pre-scaled by sqrt(D)
            etiles = []
            for v in range(V // P):
                et = sb.tile([P, D], F32, name=f"et{v}")
                nc.sync.dma_start(out=et[:], in_=emb_weight.ap()[v * P:(v + 1) * P, :])
                es = sb.tile([P, D], F32, name=f"es{v}")
                nc.any.tensor_scalar(out=es[:], in0=et[:], scalar1=SCALE,
                                     scalar2=None, op0=mybir.AluOpType.mult)
                etiles.append(es)

            # WT = out_proj_w.T via PE transpose
            wt_sb = sb.tile([P, E], F32, name="wt_sb")
            for j in range(E // P):
                wtile = sb.tile([P, P], F32, name="wtile", tag="wtile", bufs=2)
                nc.sync.dma_start(out=wtile[:], in_=out_proj_w.ap()[j * P:(j + 1) * P, :])
                pst = ps_o.tile([P, P], F32, name="pst", tag="pso")
                nc.tensor.transpose(out=pst[:], in_=wtile[:], identity=identity)
                nc.any.tensor_copy(out=wt_sb[:, j * P:(j + 1) * P], in_=pst[:])

            # ---- pipeline: gpsimd=compares+gathers, PE=matmuls,
            #      DVE=scans(+recip), ACT=psum reads; scans read PSUM directly
            SL = max(hi - lo for lo, hi in windows) + 1

            # compares early, in group order (gpsimd)
            all_ohs = []
            for g in range(NGRP):
                lo, hi = windows[g]
                L = hi - lo
                ohs = []
                for v in range(V // P):
                    oh = oh_pool.tile([P, L], F32, name="oh", tag="oh")
                    nc.any.tensor_scalar(
                        out=oh[:], in0=x16_g[g][:], scalar1=vcols[v],
                        scalar2=None, op0=mybir.AluOpType.is_equal,
                    )
                    ohs.append(oh)
                all_ohs.append(ohs)

            for g in range(NGRP):
                lo, hi = windows[g]
                L = hi - lo
                nsub = (L + 511) // 512
                ohs = all_ohs[g]

                # byte_emb^T[dim, pos] = sum_v EsT_v @ XohT_v  (PE)
                psg = ps_g.tile([P, L], F32, name="psg")
                for s in range(nsub):
                    c0, c1 = 512 * s, min(512 * (s + 1), L)
                    for v in range(V // P):
                        nc.tensor.matmul(
                            out=psg[:, c0:c1], lhsT=etiles[v][:],
                            rhs=ohs[v][:, c0:c1],
                            start=(v == 0), stop=(v == V // P - 1),
                        )

                # exclusive prefix sums straight out of PSUM (DVE)
                scg = sc_pool.tile([P, SL], F32, name="scg")
                nc.vector.memset(scg[:, 0:1], 0.0)
                if L + 1 < SL:
                    nc.vector.memset(scg[:, 1 + L:SL], 0.0)
                nc.vector.tensor_tensor_scan(
                    out=scg[:, 1:1 + L], data0=psg[:],
                    data1=zcol[:].to_broadcast([P, L]),
                    initial=0.0, op0=mybir.AluOpType.add,
                    op1=mybir.AluOpType.add,
                )

                # per-group boundary gather (stock IndirectCopy on gpsimd)
                bnd = sb.tile([P, NBPAD], F32, name="bnd", tag="bnd", bufs=2)
                nc.gpsimd.indirect_copy(
                    out=bnd[:], data=scg[:], idxs=bidx_ts[g][:],
                    i_know_ap_gather_is_preferred=True,
                )

                # segment sums = bnd[t+1] - bnd[t]  (DVE/ACT via any)
                grp = sb.tile([P, TG], F32, name="grp", tag="grp", bufs=2)
                nc.any.tensor_tensor(
                    out=grp[:], in0=bnd[:, 1:NB], in1=bnd[:, 0:NB - 1],
                    op=mybir.AluOpType.subtract,
                )

                # projection + fused mean + store
                for j in range(TG // P):
                    r = g * (TG // P) + j
                    for h in range(2):
                        pso = ps_o.tile([P, 512], F32, name="pso", tag="pso")
                        nc.tensor.matmul(
                            out=pso[:], lhsT=grp[:, j * P:(j + 1) * P],
                            rhs=wt_sb[:, h * 512:(h + 1) * 512],
                            start=True, stop=True,
                        )
                        osb = outsb_pool.tile([P, 512], F32, name="osb")
                        nc.any.tensor_scalar(
                            out=osb[:], in0=pso[:],
                            scalar1=recip_pt[:, r:r + 1], scalar2=None,
                            op0=mybir.AluOpType.mult,
                        )
                        nc.sync.dma_start(
                            out=out.ap()[r * P:(r + 1) * P, h * 512:(h + 1) * 512],
                            in_=osb[:],
                        )

    nc.compile()
    return nc


def _prep_inputs(x, byte_groups, emb_weight, out_proj_w, windows, starts):
    """Host-side integer index plumbing (no float math on tensor data)."""
    in_maps = []
    for k in range(B):
        xr = np.broadcast_to(x[k].astype(np.int16), (P, S)).copy()
        sta = starts[k, :T].reshape(T // P, P).T.astype(np.int32)   # [128, 16]
        stb = starts[k, 1:T + 1].reshape(T // P, P).T.astype(np.int32)
        bxs = []
        for g in range(NGRP):
            lo, hi = windows[g]
            loc = (starts[k, TG * g:TG * (g + 1) + 1] - lo).astype(np.uint16)
            pad = np.full(NBPAD, loc[-1], np.uint16)
            pad[:NB] = loc
            w = pad.reshape(NBPAD // 16, 16).T
            bx = np.zeros((P, NBPAD // 16), np.uint16)
            for rep in range(8):
                bx[16 * rep:16 * (rep + 1), :] = w
            bxs.append(bx)
        consts = np.zeros((P, P + V // P), np.float32)
        consts[:, 0:P] = np.eye(P, dtype=np.float32)
        for v in range(V // P):
            consts[:, P + v] = v * P + np.arange(P)
        in_maps.append({
            "x_rep": xr,
            "consts": consts,
            "st_a": np.ascontiguousarray(sta),
            "st_b": np.ascontiguousarray(stb),
            **{f"bidx{g}": bxs[g] for g in range(NGRP)},
            "emb_weight": np.asarray(emb_weight, dtype=np.float32),
            "out_proj_w": np.asarray(out_proj_w, dtype=np.float32),
        })
    return in_maps


def _run(x, byte_groups, emb_weight, out_proj_w, trace=False, **kw):
    x = np.asarray(x)
    byte_groups = np.asarray(byte_groups)
    starts = np.stack(
        [np.searchsorted(byte_groups[k], np.arange(T + 1)) for k in range(B)]
    )
    windows = _windows(starts)
    nc = _build(windows)
    in_maps = _prep_inputs(x, byte_groups, emb_weight, out_proj_w, windows, starts)
    res = run_bass_kernel_spmd(nc, in_maps, core_ids=list(range(B)), trace=trace, **kw)
    outs = np.stack([res.results[k]["out"] for k in range(B)], axis=0)
    return outs, res


def kernel(x, byte_groups, emb_weight, out_proj_w):
    outs, _ = _run(x, byte_groups, emb_weight, out_proj_w, trace=False)
    return outs



# revision 1
# speedup vs baseline: 1.8600x; 1.8600x over previous
"""Trainium2 Bass kernel for nn_ByteEmbedding (segment_reduce).

Computation (per batch row, one row per NeuronCore, 8 cores):
  byte_emb = emb_weight[x] * sqrt(128)            # gather  [8192, 128]
  grouped  = segment_mean(byte_emb, byte_groups)  # ragged  [2048, 128]
  out      = grouped @ out_proj_w.T               # proj    [2048, 1024]

Device pipeline (scan formulation, transposed layout [dim, pos]):
  1. One-hot vocab rows XohT[v, i] = (x[i] == v) built on DVE/ACT from a
     replicated copy of x (3 chunks of 128 vocab rows).
  2. byte_emb^T = (E*sqrt(128))^T @ XohT on the PE -- the embedding
     "gather" is 3 accumulating matmuls per 512-position slice with the
     128x128 embedding chunks stationary.  No DMA gather at all.
  3. Exclusive prefix sums along positions (tensor_tensor_scan) per
     512-token group window; segment sums are differences of the scan at
     host-precomputed segment boundary positions (ap_gather along the
     free dim).  Both boundaries of every token lie in its group window,
     so the scan carry cancels in the difference and the four group
     scans are fully independent (run on DVE and GpSimd in parallel).
  4. Mean: counts = diff of boundary positions (computed on device from
     the uploaded integer starts vector), reciprocal on the Scalar
     engine (activation table), multiplied into the grouped values.
  5. out = grouped^T.T @ W^T per 128-token tile (W^T built once on the
     PE via transposes).

The group windows are the union over the 8 rows, so one SPMD program
serves all cores; per-core behavior enters only through uploaded integer
index tensors (replicated x, boundary positions).
"""

import os
import sys

import numpy as np

for _p in ("/opt/trn_rl_repo",):
    if _p not in sys.path and os.path.isdir(_p):
        sys.path.append(_p)

import concourse.bacc as bacc
import concourse.bass as bass
import concourse.mybir as mybir
import concourse.tile as tile
from concourse.bass_utils import run_bass_kernel_spmd

B = 8
S = 8192          # bytes per row
V = 384           # vocab (= 3 * 128)
D = 128           # byte dim
E = 1024          # out dim
T = 2048          # tokens
P = 128
NGRP = 4          # token groups of 512
TG = T // NGRP    # 512 tokens per group
NB = TG + 1       # boundaries per group (inclusive)
NBPAD = 528       # padded boundary count (% 16 == 0, % 4 == 0)
SCALE = float(D) ** 0.5
dt = mybir.dt
F32 = dt.float32


def _windows(starts):
    """Union [lo, hi) position window per 512-token group over all rows."""
    w = []
    for g in range(NGRP):
        lo = int(starts[:, TG * g].min())
        hi = int(starts[:, TG * (g + 1)].max())
        w.append((lo, hi))
    return w


def _build(windows) -> bacc.Bacc:
    nc = bacc.Bacc(
        "TRN2",
        target_bir_lowering=False,
        debug=False,
        enable_asserts=True,
        num_devices=B,
    )

    x_rep = nc.dram_tensor("x_rep", [P, S], dt.int16, kind="ExternalInput")
    st_a = nc.dram_tensor("st_a", [P, T // P], dt.int32, kind="ExternalInput")
    st_b = nc.dram_tensor("st_b", [P, T // P], dt.int32, kind="ExternalInput")
    bidxs = [nc.dram_tensor(f"bidx{g}", [P, NBPAD // 16], dt.uint16, kind="ExternalInput")
             for g in range(NGRP)]
    consts = nc.dram_tensor("consts", [P, P + V // P], F32, kind="ExternalInput")
    emb_weight = nc.dram_tensor("emb_weight", [V, D], F32, kind="ExternalInput")
    out_proj_w = nc.dram_tensor("out_proj_w", [E, D], F32, kind="ExternalInput")
    out = nc.dram_tensor("out", [T, E], F32, kind="ExternalOutput")

    with tile.TileContext(nc) as tc:
        with (
            tc.tile_pool(name="sb", bufs=1) as sb,
            tc.tile_pool(name="oh", bufs=6) as oh_pool,
            tc.tile_pool(name="bem", bufs=2) as bem_pool,
            tc.tile_pool(name="sc", bufs=2) as sc_pool,
            tc.tile_pool(name="outsb", bufs=4) as outsb_pool,
            tc.tile_pool(name="ps_g", bufs=1, space="PSUM") as ps_g,
            tc.tile_pool(name="ps_o", bufs=3, space="PSUM") as ps_o,
        ):
            # ---- inputs: per-window slices of the replicated x row ----
            x16_g = []
            for g in range(NGRP):
                lo, hi = windows[g]
                xg = sb.tile([P, hi - lo], dt.int16, name=f"x16_{g}")
                nc.sync.dma_start(out=xg[:], in_=x_rep.ap()[:, lo:hi])
                x16_g.append(xg)

            bidx_ts = []
            for g in range(NGRP):
                bt = sb.tile([P, NBPAD // 16], dt.uint16, name=f"bidx_t{g}")
                nc.sync.dma_start(out=bt[:], in_=bidxs[g].ap())
                bidx_ts.append(bt)

            # ---- reciprocal counts in token-partition layout ----
            # recip_pt[p, c] = 1 / max(starts[128c+p+1] - starts[128c+p], 1);
            # applied after the projection (out rows are tokens there).
            sta_i = sb.tile([P, T // P], dt.int32, name="sta_i")
            nc.sync.dma_start(out=sta_i[:], in_=st_a.ap())
            stb_i = sb.tile([P, T // P], dt.int32, name="stb_i")
            nc.sync.dma_start(out=stb_i[:], in_=st_b.ap())
            sta_f = sb.tile([P, T // P], F32, name="sta_f")
            nc.vector.tensor_copy(out=sta_f[:], in_=sta_i[:])
            stb_f = sb.tile([P, T // P], F32, name="stb_f")
            nc.vector.tensor_copy(out=stb_f[:], in_=stb_i[:])
            cnt_pt = sb.tile([P, T // P], F32, name="cnt_pt")
            nc.vector.tensor_tensor(out=cnt_pt[:], in0=stb_f[:], in1=sta_f[:],
                                    op=mybir.AluOpType.subtract)
            nc.vector.tensor_scalar(out=cnt_pt[:], in0=cnt_pt[:], scalar1=1.0,
                                    scalar2=None, op0=mybir.AluOpType.max)
            recip_pt = sb.tile([P, T // P], F32, name="recip_pt")
            nc.vector.reciprocal(out=recip_pt[:], in_=cnt_pt[:])

            # ---- constants (uploaded: keeps gpsimd gather-only => one
            #      library reload, no per-group reload drains) ----
            consts_t = sb.tile([P, P + V // P], F32, name="consts_t")
            nc.sync.dma_start(out=consts_t[:], in_=consts.ap())
            identity = consts_t[:, 0:P]
            vcols = [consts_t[:, P + v:P + v + 1] for v in range(V // P)]
            zcol = sb.tile([P, 1], F32, name="zcol")
            nc.vector.memset(zcol[:], 0.0)

            # embedding chunks, pre-scaled by sqrt(D)
            etiles = []
            for v in range(V // P):
                et = sb.tile([P, D], F32, name=f"et{v}")
                nc.sync.dma_start(out=et[:], in_=emb_weight.ap()[v * P:(v + 1) * P, :])
                es = sb.tile([P, D], F32, name=f"es{v}")
                nc.any.tensor_scalar(out=es[:], in0=et[:], scalar1=SCALE,
                                     scalar2=None, op0=mybir.AluOpType.mult)
                etiles.append(es)

            # WT = out_proj_w.T via PE transpose
            wt_sb = sb.tile([P, E], F32, name="wt_sb")
            for j in range(E // P):
                wtile = sb.tile([P, P], F32, name="wtile", tag="wtile", bufs=2)
                nc.sync.dma_start(out=wtile[:], in_=out_proj_w.ap()[j * P:(j + 1) * P, :])
                pst = ps_o.tile([P, P], F32, name="pst", tag="pso")
                nc.tensor.transpose(out=pst[:], in_=wtile[:], identity=identity)
                nc.any.tensor_copy(out=wt_sb[:, j * P:(j + 1) * P], in_=pst[:])

            # ---- pipeline: gpsimd=compares+gathers, PE=matmuls,
            #      DVE=scans(+recip), ACT=psum reads; scans read PSUM directly
            SL = max(hi - lo for lo, hi in windows) + 1

            # compares early, in group order (gpsimd)
            all_ohs = []
            for g in range(NGRP):
                lo, hi = windows[g]
                L = hi - lo
                ohs = []
                for v in range(V // P):
                    oh = oh_pool.tile([P, L], F32, name="oh", tag="oh")
                    nc.any.tensor_scalar(
                        out=oh[:], in0=x16_g[g][:], scalar1=vcols[v],
                        scalar2=None, op0=mybir.AluOpType.is_equal,
                    )
                    ohs.append(oh)
                all_ohs.append(ohs)

            for g in range(NGRP):
                lo, hi = windows[g]
                L = hi - lo
                nsub = (L + 511) // 512
                ohs = all_ohs[g]

                # byte_emb^T[dim, pos] = sum_v EsT_v @ XohT_v  (PE)
                psg = ps_g.tile([P, L], F32, name="psg")
                for s in range(nsub):
                    c0, c1 = 512 * s, min(512 * (s + 1), L)
                    for v in range(V // P):
                        nc.tensor.matmul(
                            out=psg[:, c0:c1], lhsT=etiles[v][:],
                            rhs=ohs[v][:, c0:c1],
                            start=(v == 0), stop=(v == V // P - 1),
                        )

                # exclusive prefix sums straight out of PSUM (DVE)
                scg = sc_pool.tile([P, SL], F32, name="scg")
                nc.vector.memset(scg[:, 0:1], 0.0)
                if L + 1 < SL:
                    nc.vector.memset(scg[:, 1 + L:SL], 0.0)
                nc.vector.tensor_tensor_scan(
                    out=scg[:, 1:1 + L], data0=psg[:],
                    data1=zcol[:].to_broadcast([P, L]),
                    initial=0.0, op0=mybir.AluOpType.add,
                    op1=mybir.AluOpType.add,
                )

                # per-group boundary gather (stock IndirectCopy on gpsimd)
                bnd = sb.tile([P, NBPAD], F32, name="bnd", tag="bnd", bufs=2)
                nc.gpsimd.indirect_copy(
                    out=bnd[:], data=scg[:], idxs=bidx_ts[g][:],
                    i_know_ap_gather_is_preferred=True,
                )

                # segment sums = bnd[t+1] - bnd[t]  (DVE/ACT via any)
                grp = sb.tile([P, TG], F32, name="grp", tag="grp", bufs=2)
                nc.any.tensor_tensor(
                    out=grp[:], in0=bnd[:, 1:NB], in1=bnd[:, 0:NB - 1],
                    op=mybir.AluOpType.subtract,
                )

                # projection + fused mean + store
                for j in range(TG // P):
                    r = g * (TG // P) + j
                    for h in range(2):
                        pso = ps_o.tile([P, 512], F32, name="pso", tag="pso")
                        nc.tensor.matmul(
                            out=pso[:], lhsT=grp[:, j * P:(j + 1) * P],
                            rhs=wt_sb[:, h * 512:(h + 1) * 512],
                            start=True, stop=True,
                        )
                        osb = outsb_pool.tile([P, 512], F32, name="osb")
                        nc.any.tensor_scalar(
                            out=osb[:], in0=pso[:],
                            scalar1=recip_pt[:, r:r + 1], scalar2=None,
                            op0=mybir.AluOpType.mult,
                        )
                        nc.sync.dma_start(
                            out=out.ap()[r * P:(r + 1) * P, h * 512:(h + 1) * 512],
                            in_=osb[:],
                        )

    nc.compile()
    return nc


def _prep_inputs(x, byte_groups, emb_weight, out_proj_w, windows, starts):
    """Host-side integer index plumbing (no float math on tensor data)."""
    in_maps = []
    for k in range(B):
        xr = np.broadcast_to(x[k].astype(np.int16), (P, S)).copy()
        sta = starts[k, :T].reshape(T // P, P).T.astype(np.int32)   # [128, 16]
        stb = starts[k, 1:T + 1].reshape(T // P, P).T.astype(np.int32)
        bxs = []
        for g in range(NGRP):
            lo, hi = windows[g]
            loc = (starts[k, TG * g:TG * (g + 1) + 1] - lo).astype(np.uint16)
            pad = np.full(NBPAD, loc[-1], np.uint16)
            pad[:NB] = loc
            w = pad.reshape(NBPAD // 16, 16).T
            bx = np.zeros((P, NBPAD // 16), np.uint16)
            for rep in range(8):
                bx[16 * rep:16 * (rep + 1), :] = w
            bxs.append(bx)
        consts = np.zeros((P, P + V // P), np.float32)
        consts[:, 0:P] = np.eye(P, dtype=np.float32)
        for v in range(V // P):
            consts[:, P + v] = v * P + np.arange(P)
        in_maps.append({
            "x_rep": xr,
            "consts": consts,
            "st_a": np.ascontiguousarray(sta),
            "st_b": np.ascontiguousarray(stb),
            **{f"bidx{g}": bxs[g] for g in range(NGRP)},
            "emb_weight": np.asarray(emb_weight, dtype=np.float32),
            "out_proj_w": np.asarray(out_proj_w, dtype=np.float32),
        })
    return in_maps


def _run(x, byte_groups, emb_weight, out_proj_w, trace=False, **kw):
    x = np.asarray(x)
    byte_groups = np.asarray(byte_groups)
    starts = np.stack(
        [np.searchsorted(byte_groups[k], np.arange(T + 1)) for k in range(B)]
    )
    windows = _windows(starts)
    nc = _build(windows)
    in_maps = _prep_inputs(x, byte_groups, emb_weight, out_proj_w, windows, starts)
    res = run_bass_kernel_spmd(nc, in_maps, core_ids=list(range(B)), trace=trace, **kw)
    outs = np.stack([res.results[k]["out"] for k in range(B)], axis=0)
    return outs, res


def kernel(x, byte_groups, emb_weight, out_proj_w):
    outs, _ = _run(x, byte_groups, emb_weight, out_proj_w, trace=False)
    return outs

